# revision 18
# baseline (speedup 1.0000x reference)
"""TRN2 Bass kernel for nn_ClassAttention (1x1 conv + BN + ReLU + windowed attention).

kernel(**inputs) takes FULL inputs, returns the FULL output [4,256,256,256] f32.
Shards data-parallel over (batch, image-row-half) across 8 NeuronCores, runs a
Bass/Tile SPMD program via run_bass_kernel_spmd, and unshards on the host.

All bulk tensors are staged in bf16 (host converts): halves HBM traffic vs f32
(96 MiB/core) and runs the PE at 1 cycle/row. PSUM accumulation stays f32.
Elementwise work (bias add, relu, evac casts) is batched over groups of 4
window-pairs so the fixed per-instruction overheads (~150ns) amortize over
1024-elem free dims instead of 256.

Per-core shard (core = (b, rh) = (core//2, core%2)):
  x_sh     [256c, 16hh, 2048]  bf16, x[b,:,128rh:+128,:] window-contiguous:
                               [c, hh, (pw, win, r1, r2)]
  at_sh    [16hh, 128, 16384]  bf16, attn pre-transposed [pair, 64*win+k, 64*nh+q],
                               partition-major per row of windows
  w_prep   [256c, 256o]        bf16, (w_conv * inv_std[:,None]).T  (BN scale folded)
  bias4    [128, 1024]         bf16, (beta - mean*inv_std) tiled 4x across pairs
  out      [16hh, 128p, 4096]  bf16 staging dump; host decodes
                               p = 32quad+16win+d, f = pw*256 + j*64 + r1*8 + r2,
                               nh = 4j+quad, ch = 16nh+d

On-chip pipeline per group of 4 window-pairs (pair = 2 windows of 64 pixels,
pixels on partitions):
  conv (PE): ps4[128pix=(win,r1,r2), 1024=(pair,nh,d)] = x.T @ w_prep, 8 matmuls
  bias (DVE): tv4 = ps4 + bias4, one [128,1024] f32 instr per group
  relu (ACT): block-diagonal V4 [128, (pair,nh,win,d)] bf16: ONE instr per
        window half covers 4 pairs (4D strided dest); off-diag cells stay zero
        (zeroed once at start, never rewritten)
  attn (PE): per (pair, head) one matmul computes BOTH windows via block-diag:
             out[32,64] = V4[:,512iG+32nh:+32].T @ At[:,64nh:+64], K=128, N=64,
             tile_position=(0, 32*(nh%4)) -> 4 column-tiles packed in the array
  evac (DVE): pa4 psum [128, 1024] f32 -> staging bf16, one CAST per group
  store (ACT hwdge ring): staging -> DRAM in 256 KiB per-group chunks
"""

import numpy as np
from contextlib import ExitStack

import ml_dtypes

import concourse.bacc as bacc
import concourse.tile as tile
import concourse.mybir as mybir
from concourse.bass_utils import run_bass_kernel_spmd

F32 = mybir.dt.float32
BF16 = mybir.dt.bfloat16
RELU = mybir.ActivationFunctionType.Relu
NP_BF16 = ml_dtypes.bfloat16

EPS = 1e-5
NCORES = 8

_cached_nc = None


def _build_program(n_vbd=6, at_bufs=8, G=4):
    nc = bacc.Bacc("TRN2", target_bir_lowering=False, debug=False)

    x_d = nc.dram_tensor("x_sh", [256, 16, 2048], BF16, kind="ExternalInput")
    at_d = nc.dram_tensor("at_sh", [16, 128, 16384], BF16, kind="ExternalInput")
    wc_d = nc.dram_tensor("w_prep", [256, 256], BF16, kind="ExternalInput")
    b_d = nc.dram_tensor("bias4", [128, 1024], BF16, kind="ExternalInput")
    out_d = nc.dram_tensor("out_sh", [16, 128, 4096], BF16, kind="ExternalOutput")

    ngroups = 16 // G

    with tile.TileContext(nc) as tc, ExitStack() as ctx:
        const = ctx.enter_context(tc.tile_pool(name="const", bufs=1))
        xp = ctx.enter_context(tc.tile_pool(name="xp", bufs=4))
        atp = ctx.enter_context(tc.tile_pool(name="atp", bufs=at_bufs))
        vbdp = ctx.enter_context(tc.tile_pool(name="vbdp", bufs=1))
        tvp = ctx.enter_context(tc.tile_pool(name="tvp", bufs=3))
        stp = ctx.enter_context(tc.tile_pool(name="stp", bufs=3))
        pscp = ctx.enter_context(tc.tile_pool(name="pscp", bufs=2, space="PSUM"))
        psap = ctx.enter_context(tc.tile_pool(name="psap", bufs=2, space="PSUM"))

        w0 = const.tile([128, 256], BF16, name="w0")
        w1 = const.tile([128, 256], BF16, name="w1")
        nc.sync.dma_start(out=w0, in_=wc_d[0:128, :])
        nc.sync.dma_start(out=w1, in_=wc_d[128:256, :])
        bias4 = const.tile([128, 1024], BF16, name="bias4_t")
        nc.gpsimd.dma_start(out=bias4, in_=b_d[:, :])

        # Block-diagonal V4 tiles: columns = (pair 4, nh 16, win 2, d 16).
        # Zeroed once; relu writes only the diagonal cells (win0 -> rows 0:64
        # of win-0 columns, win1 -> rows 64:128 of win-1 columns), so zeros
        # persist across reuse and each V4[:, 512iG+32nh:+32] is exactly
        # block-diag(V0, V1) for pair iG head nh.
        vbd = []
        for i in range(n_vbd):
            t = vbdp.tile([128, 2048], BF16, tag=f"vbd{i}", name=f"vbd{i}")
            nc.vector.memset(t, 0.0)
            vbd.append(t)
        vbd_i = 0

        for hh in range(16):
            xt0 = xp.tile([128, 2048], BF16, tag="xt0", name=f"xt0_{hh}")
            xt1 = xp.tile([128, 2048], BF16, tag="xt1", name=f"xt1_{hh}")
            nc.gpsimd.dma_start(out=xt0, in_=x_d[0:128, hh, :])
            nc.gpsimd.dma_start(out=xt1, in_=x_d[128:256, hh, :])

            st = stp.tile([128, 4096], BF16, tag="st", name=f"st_{hh}")

            for g in range(ngroups):
                at = atp.tile([128, 1024 * G], BF16, tag="at", name=f"at_{hh}_{g}")
                nc.sync.dma_start(
                    out=at,
                    in_=at_d[hh, :, 1024 * G * g: 1024 * G * (g + 1)])

                # conv: 4 pairs into one grouped psum tile
                ps4 = pscp.tile([128, 1024], F32, tag="psc", name=f"ps_{hh}_{g}")
                for iG in range(G):
                    p8 = G * g + iG
                    xsl = slice(128 * p8, 128 * p8 + 128)
                    osl = slice(256 * iG, 256 * iG + 256)
                    nc.tensor.matmul(ps4[:, osl], xt0[:, xsl], w0,
                                     start=True, stop=False)
                    nc.tensor.matmul(ps4[:, osl], xt1[:, xsl], w1,
                                     start=False, stop=True)

                # bias add, one batched instr (DVE), psum -> sbuf f32
                tv4 = tvp.tile([128, 1024], F32, tag="tv", name=f"tv_{hh}_{g}")
                nc.vector.tensor_add(tv4, ps4, bias4)

                # relu into block-diag V4, one batched instr per window half
                V4 = vbd[vbd_i % n_vbd]
                vbd_i += 1
                V4r = V4.rearrange("p (iG nh two d) -> p iG nh two d",
                                   iG=4, nh=16, two=2, d=16)
                tv4r = tv4.rearrange("p (iG nh d) -> p iG nh d",
                                     iG=4, nh=16, d=16)
                nc.scalar.activation(V4r[0:64, :, :, 0, :], tv4r[0:64], RELU)
                nc.scalar.activation(V4r[64:128, :, :, 1, :], tv4r[64:128], RELU)

                # attention: 16 matmuls per pair into grouped psum
                pa4 = psap.tile([128, 1024], F32, tag="pa", name=f"pa_{hh}_{g}")
                for iG in range(G):
                    for j in range(4):
                        for quad in range(4):
                            nh = 4 * j + quad
                            nc.tensor.matmul(
                                pa4[32 * quad:32 * quad + 32,
                                    256 * iG + 64 * j: 256 * iG + 64 * j + 64],
                                V4[:, 512 * iG + 32 * nh: 512 * iG + 32 * nh + 32],
                                at[:, 1024 * iG + 64 * nh: 1024 * iG + 64 * nh + 64],
                                start=True, stop=True,
                                tile_position=(0, 32 * quad))

                # evac: one batched CAST (DVE), then store this 256KiB chunk
                nc.vector.tensor_copy(st[:, 1024 * g:1024 * (g + 1)], pa4)
                nc.scalar.dma_start(out=out_d[hh][:, 1024 * g:1024 * (g + 1)],
                                    in_=st[:, 1024 * g:1024 * (g + 1)])

    nc.compile()
    return nc


def _shard_inputs(x, attn_i, w_conv, bn_gamma, bn_beta, bn_mean, bn_var):
    inv_std = (bn_gamma / np.sqrt(bn_var + np.float32(EPS))).astype(np.float32)
    shift = (bn_beta - bn_mean * inv_std).astype(np.float32)
    bias4 = np.ascontiguousarray(
        np.broadcast_to(np.tile(shift, 4)[None, :], (128, 1024))).astype(NP_BF16)
    w_prep = np.ascontiguousarray((w_conv * inv_std[:, None]).T).astype(NP_BF16)
    x16 = x.astype(NP_BF16)
    at16 = attn_i.astype(NP_BF16)
    in_maps = []
    for core in range(NCORES):
        b, rh = core // 2, core % 2
        x_sh = x16[b, :, 128 * rh:128 * rh + 128, :]
        x_sh = np.ascontiguousarray(
            x_sh.reshape(256, 16, 8, 16, 2, 8).transpose(0, 1, 3, 4, 2, 5)
        ).reshape(256, 16, 2048)
        a_sl = at16[1024 * b + 512 * rh: 1024 * b + 512 * rh + 512]
        # [pair, 64win+k, 64nh+q], then partition-major per hh row
        # ([hh, p, pr, 1024]) so each at-load reads 8KiB/partition contiguous
        a_prep = a_sl.reshape(256, 2, 16, 64, 64).transpose(0, 1, 4, 2, 3) \
            .reshape(16, 16, 128, 1024)
        a_prep = np.ascontiguousarray(
            a_prep.transpose(0, 2, 1, 3)).reshape(16, 128, 16384)
        in_maps.append(dict(x_sh=x_sh, at_sh=a_prep, w_prep=w_prep,
                            bias4=bias4))
    return in_maps


def _unshard_output(results):
    out = np.empty((4, 256, 256, 256), np.float32)
    for core in range(NCORES):
        b, rh = core // 2, core % 2
        raw = np.asarray(results[core]["out_sh"]).astype(np.float32)
        # [hh, (quad, win, d), (pw, j, r1, r2)],  nh = 4j+quad
        r = raw.reshape(16, 4, 2, 16, 16, 4, 8, 8)
        # ch = 64j+16quad+d ; h = 8hh+r1 ; w = 16pw+8win+r2
        oc = r.transpose(5, 1, 3, 0, 6, 4, 2, 7).reshape(256, 128, 256)
        out[b, :, 128 * rh:128 * rh + 128, :] = oc
    return out


def get_program():
    global _cached_nc
    if _cached_nc is None:
        _cached_nc = _build_program()
    return _cached_nc


def run_sharded(in_maps, trace=False, **kwargs):
    nc = get_program()
    return run_bass_kernel_spmd(nc, in_maps, list(range(NCORES)),
                                trace=trace, **kwargs)


def kernel(x, attn_i, w_conv, bn_gamma, bn_beta, bn_mean, bn_var):
    x = np.asarray(x, dtype=np.float32)
    attn_i = np.asarray(attn_i, dtype=np.float32)
    w_conv = np.asarray(w_conv, dtype=np.float32)
    bn_gamma = np.asarray(bn_gamma, dtype=np.float32)
    bn_beta = np.asarray(bn_beta, dtype=np.float32)
    bn_mean = np.asarray(bn_mean, dtype=np.float32)
    bn_var = np.asarray(bn_var, dtype=np.float32)
    in_maps = _shard_inputs(x, attn_i, w_conv, bn_gamma, bn_beta, bn_mean, bn_var)
    res = run_sharded(in_maps)
    return _unshard_output(res.results)


# revision 19
# speedup vs baseline: 1.0095x; 1.0095x over previous
"""TRN2 Bass kernel for nn_ClassAttention (1x1 conv + BN + ReLU + windowed attention).

kernel(**inputs) takes FULL inputs, returns the FULL output [4,256,256,256] f32.
Shards data-parallel over (batch, image-row-half) across 8 NeuronCores, runs a
Bass/Tile SPMD program via run_bass_kernel_spmd, and unshards on the host.

All bulk tensors are staged in bf16 (host converts): halves HBM traffic vs f32
(96 MiB/core) and runs the PE at 1 cycle/row. PSUM accumulation stays f32.
Elementwise work (bias add, relu, evac casts) is batched over groups of 4
window-pairs so the fixed per-instruction overheads (~150ns) amortize over
1024-elem free dims instead of 256.

Per-core shard (core = (b, rh) = (core//2, core%2)):
  x_sh     [256c, 16hh, 2048]  bf16, x[b,:,128rh:+128,:] window-contiguous:
                               [c, hh, (pw, win, r1, r2)]
  at_sh    [16hh, 128, 16384]  bf16, attn pre-transposed [pair, 64*win+k, 64*nh+q],
                               partition-major per row of windows
  w_prep   [256c, 256o]        bf16, (w_conv * inv_std[:,None]).T  (BN scale folded)
  bias4    [128, 1024]         f32, (beta - mean*inv_std) tiled 4x across pairs
  out      [16hh, 128p, 4096]  bf16 staging dump; host decodes
                               p = 32quad+16win+d, f = pw*256 + j*64 + r1*8 + r2,
                               nh = 4j+quad, ch = 16nh+d

On-chip pipeline per group of 4 window-pairs (pair = 2 windows of 64 pixels,
pixels on partitions):
  conv (PE): ps4[128pix=(win,r1,r2), 1024=(pair,nh,d)] = x.T @ w_prep, 8 matmuls
  bias (DVE): tv4 = ps4 + bias4, one [128,1024] f32 instr per group
  relu (ACT): block-diagonal V4 [128, (pair,nh,win,d)] bf16: ONE instr per
        window half covers 4 pairs (4D strided dest); off-diag cells stay zero
        (zeroed once at start, never rewritten)
  attn (PE): per (pair, head) one matmul computes BOTH windows via block-diag:
             out[32,64] = V4[:,512iG+32nh:+32].T @ At[:,64nh:+64], K=128, N=64,
             tile_position=(0, 32*(nh%4)) -> 4 column-tiles packed in the array
  evac (DVE): pa4 psum [128, 1024] f32 -> staging bf16, one CAST per group
  store (ACT hwdge ring): staging -> DRAM in 256 KiB per-group chunks
"""

import numpy as np
from contextlib import ExitStack

import ml_dtypes

import concourse.bacc as bacc
import concourse.tile as tile
import concourse.mybir as mybir
from concourse.bass_utils import run_bass_kernel_spmd

F32 = mybir.dt.float32
BF16 = mybir.dt.bfloat16
RELU = mybir.ActivationFunctionType.Relu
NP_BF16 = ml_dtypes.bfloat16

EPS = 1e-5
NCORES = 8

_cached_nc = None


def _build_program(n_vbd=6, at_bufs=8, G=4):
    nc = bacc.Bacc("TRN2", target_bir_lowering=False, debug=False)

    x_d = nc.dram_tensor("x_sh", [256, 16, 2048], BF16, kind="ExternalInput")
    at_d = nc.dram_tensor("at_sh", [16, 128, 16384], BF16, kind="ExternalInput")
    wc_d = nc.dram_tensor("w_prep", [256, 256], BF16, kind="ExternalInput")
    b_d = nc.dram_tensor("bias4", [128, 1024], F32, kind="ExternalInput")
    out_d = nc.dram_tensor("out_sh", [16, 128, 4096], BF16, kind="ExternalOutput")

    ngroups = 16 // G

    with tile.TileContext(nc) as tc, ExitStack() as ctx:
        const = ctx.enter_context(tc.tile_pool(name="const", bufs=1))
        xp = ctx.enter_context(tc.tile_pool(name="xp", bufs=4))
        atp = ctx.enter_context(tc.tile_pool(name="atp", bufs=at_bufs))
        vbdp = ctx.enter_context(tc.tile_pool(name="vbdp", bufs=1))
        tvp = ctx.enter_context(tc.tile_pool(name="tvp", bufs=3))
        stp = ctx.enter_context(tc.tile_pool(name="stp", bufs=3))
        pscp = ctx.enter_context(tc.tile_pool(name="pscp", bufs=2, space="PSUM"))
        psap = ctx.enter_context(tc.tile_pool(name="psap", bufs=2, space="PSUM"))

        w0 = const.tile([128, 256], BF16, name="w0")
        w1 = const.tile([128, 256], BF16, name="w1")
        nc.sync.dma_start(out=w0, in_=wc_d[0:128, :])
        nc.sync.dma_start(out=w1, in_=wc_d[128:256, :])
        bias4 = const.tile([128, 1024], F32, name="bias4_t")
        nc.sync.dma_start(out=bias4, in_=b_d[:, :])

        # Block-diagonal V4 tiles: columns = (pair 4, nh 16, win 2, d 16).
        # Zeroed once; relu writes only the diagonal cells (win0 -> rows 0:64
        # of win-0 columns, win1 -> rows 64:128 of win-1 columns), so zeros
        # persist across reuse and each V4[:, 512iG+32nh:+32] is exactly
        # block-diag(V0, V1) for pair iG head nh.
        vbd = []
        for i in range(n_vbd):
            t = vbdp.tile([128, 2048], BF16, tag=f"vbd{i}", name=f"vbd{i}")
            nc.vector.memset(t, 0.0)
            vbd.append(t)
        vbd_i = 0

        for hh in range(16):
            xt0 = xp.tile([128, 2048], BF16, tag="xt0", name=f"xt0_{hh}")
            xt1 = xp.tile([128, 2048], BF16, tag="xt1", name=f"xt1_{hh}")
            nc.sync.dma_start(out=xt0, in_=x_d[0:128, hh, :])
            nc.sync.dma_start(out=xt1, in_=x_d[128:256, hh, :])

            st = stp.tile([128, 4096], BF16, tag="st", name=f"st_{hh}")

            for g in range(ngroups):
                at = atp.tile([128, 1024 * G], BF16, tag="at", name=f"at_{hh}_{g}")
                nc.sync.dma_start(
                    out=at,
                    in_=at_d[hh, :, 1024 * G * g: 1024 * G * (g + 1)])

                # conv: 4 pairs into one grouped psum tile
                ps4 = pscp.tile([128, 1024], F32, tag="psc", name=f"ps_{hh}_{g}")
                for iG in range(G):
                    p8 = G * g + iG
                    xsl = slice(128 * p8, 128 * p8 + 128)
                    osl = slice(256 * iG, 256 * iG + 256)
                    nc.tensor.matmul(ps4[:, osl], xt0[:, xsl], w0,
                                     start=True, stop=False)
                    nc.tensor.matmul(ps4[:, osl], xt1[:, xsl], w1,
                                     start=False, stop=True)

                # bias add, one batched instr (DVE), psum -> sbuf f32
                tv4 = tvp.tile([128, 1024], F32, tag="tv", name=f"tv_{hh}_{g}")
                nc.vector.tensor_add(tv4, ps4, bias4)

                # relu into block-diag V4, one batched instr per window half
                V4 = vbd[vbd_i % n_vbd]
                vbd_i += 1
                V4r = V4.rearrange("p (iG nh two d) -> p iG nh two d",
                                   iG=4, nh=16, two=2, d=16)
                tv4r = tv4.rearrange("p (iG nh d) -> p iG nh d",
                                     iG=4, nh=16, d=16)
                nc.scalar.activation(V4r[0:64, :, :, 0, :], tv4r[0:64], RELU)
                nc.scalar.activation(V4r[64:128, :, :, 1, :], tv4r[64:128], RELU)

                # attention: 16 matmuls per pair into grouped psum
                pa4 = psap.tile([128, 1024], F32, tag="pa", name=f"pa_{hh}_{g}")
                for iG in range(G):
                    for j in range(4):
                        for quad in range(4):
                            nh = 4 * j + quad
                            nc.tensor.matmul(
                                pa4[32 * quad:32 * quad + 32,
                                    256 * iG + 64 * j: 256 * iG + 64 * j + 64],
                                V4[:, 512 * iG + 32 * nh: 512 * iG + 32 * nh + 32],
                                at[:, 1024 * iG + 64 * nh: 1024 * iG + 64 * nh + 64],
                                start=True, stop=True,
                                tile_position=(0, 32 * quad))

                # evac: one batched CAST (DVE), then store this 256KiB chunk
                nc.vector.tensor_copy(st[:, 1024 * g:1024 * (g + 1)], pa4)
                nc.scalar.dma_start(out=out_d[hh][:, 1024 * g:1024 * (g + 1)],
                                    in_=st[:, 1024 * g:1024 * (g + 1)])

    nc.compile()
    return nc


def _shard_inputs(x, attn_i, w_conv, bn_gamma, bn_beta, bn_mean, bn_var):
    inv_std = (bn_gamma / np.sqrt(bn_var + np.float32(EPS))).astype(np.float32)
    shift = (bn_beta - bn_mean * inv_std).astype(np.float32)
    bias4 = np.ascontiguousarray(
        np.broadcast_to(np.tile(shift, 4)[None, :], (128, 1024))).astype(np.float32)
    w_prep = np.ascontiguousarray((w_conv * inv_std[:, None]).T).astype(NP_BF16)
    x16 = x.astype(NP_BF16)
    at16 = attn_i.astype(NP_BF16)
    in_maps = []
    for core in range(NCORES):
        b, rh = core // 2, core % 2
        x_sh = x16[b, :, 128 * rh:128 * rh + 128, :]
        x_sh = np.ascontiguousarray(
            x_sh.reshape(256, 16, 8, 16, 2, 8).transpose(0, 1, 3, 4, 2, 5)
        ).reshape(256, 16, 2048)
        a_sl = at16[1024 * b + 512 * rh: 1024 * b + 512 * rh + 512]
        # [pair, 64win+k, 64nh+q], then partition-major per hh row
        # ([hh, p, pr, 1024]) so each at-load reads 8KiB/partition contiguous
        a_prep = a_sl.reshape(256, 2, 16, 64, 64).transpose(0, 1, 4, 2, 3) \
            .reshape(16, 16, 128, 1024)
        a_prep = np.ascontiguousarray(
            a_prep.transpose(0, 2, 1, 3)).reshape(16, 128, 16384)
        in_maps.append(dict(x_sh=x_sh, at_sh=a_prep, w_prep=w_prep,
                            bias4=bias4))
    return in_maps


def _unshard_output(results):
    out = np.empty((4, 256, 256, 256), np.float32)
    for core in range(NCORES):
        b, rh = core // 2, core % 2
        raw = np.asarray(results[core]["out_sh"]).astype(np.float32)
        # [hh, (quad, win, d), (pw, j, r1, r2)],  nh = 4j+quad
        r = raw.reshape(16, 4, 2, 16, 16, 4, 8, 8)
        # ch = 64j+16quad+d ; h = 8hh+r1 ; w = 16pw+8win+r2
        oc = r.transpose(5, 1, 3, 0, 6, 4, 2, 7).reshape(256, 128, 256)
        out[b, :, 128 * rh:128 * rh + 128, :] = oc
    return out


def get_program():
    global _cached_nc
    if _cached_nc is None:
        _cached_nc = _build_program()
    return _cached_nc


def run_sharded(in_maps, trace=False, **kwargs):
    nc = get_program()
    return run_bass_kernel_spmd(nc, in_maps, list(range(NCORES)),
                                trace=trace, **kwargs)


def kernel(x, attn_i, w_conv, bn_gamma, bn_beta, bn_mean, bn_var):
    x = np.asarray(x, dtype=np.float32)
    attn_i = np.asarray(attn_i, dtype=np.float32)
    w_conv = np.asarray(w_conv, dtype=np.float32)
    bn_gamma = np.asarray(bn_gamma, dtype=np.float32)
    bn_beta = np.asarray(bn_beta, dtype=np.float32)
    bn_mean = np.asarray(bn_mean, dtype=np.float32)
    bn_var = np.asarray(bn_var, dtype=np.float32)
    in_maps = _shard_inputs(x, attn_i, w_conv, bn_gamma, bn_beta, bn_mean, bn_var)
    res = run_sharded(in_maps)
    return _unshard_output(res.results)


# revision 22
# speedup vs baseline: 1.0309x; 1.0212x over previous
"""TRN2 Bass kernel for nn_ClassAttention (1x1 conv + BN + ReLU + windowed attention).

kernel(**inputs) takes FULL inputs, returns the FULL output [4,256,256,256] f32.
Shards data-parallel over (batch, image-row-half) across 8 NeuronCores, runs a
Bass/Tile SPMD program via run_bass_kernel_spmd, and unshards on the host.

All bulk tensors are staged in bf16 (host converts): halves HBM traffic vs f32
(96 MiB/core) and runs the PE at 1 cycle/row. PSUM accumulation stays f32.
Elementwise work (bias add, relu, evac casts) is batched over groups of 4
window-pairs so the fixed per-instruction overheads (~150ns) amortize over
1024-elem free dims instead of 256.

Per-core shard (core = (b, rh) = (core//2, core%2)):
  x_sh     [256c, 16hh, 2048]  bf16, x[b,:,128rh:+128,:] window-contiguous:
                               [c, hh, (pw, win, r1, r2)]
  at_sh    [16hh, 128, 16384]  bf16, attn pre-transposed [pair, 64*win+k, 64*nh+q],
                               partition-major per row of windows
  w_prep   [256c, 256o]        bf16, (w_conv * inv_std[:,None]).T  (BN scale folded)
  bias4    [128, 1024]         f32, (beta - mean*inv_std) tiled 4x across pairs
  out      [16hh, 128p, 4096]  bf16 staging dump; host decodes
                               p = 32quad+16win+d, f = pw*256 + j*64 + r1*8 + r2,
                               nh = 4j+quad, ch = 16nh+d

On-chip pipeline per group of 4 window-pairs (pair = 2 windows of 64 pixels,
pixels on partitions):
  conv (PE): ps4[128pix=(win,r1,r2), 1024=(pair,nh,d)] = x.T @ w_prep, 8 matmuls
  bias (DVE): tv4 = ps4 + bias4, one [128,1024] f32 instr per group
  relu (ACT): block-diagonal V4 [128, (pair,nh,win,d)] bf16: ONE instr per
        window half covers 4 pairs (4D strided dest); off-diag cells stay zero
        (zeroed once at start, never rewritten)
  attn (PE): per (pair, head) one matmul computes BOTH windows via block-diag:
             out[32,64] = V4[:,512iG+32nh:+32].T @ At[:,64nh:+64], K=128, N=64,
             tile_position=(0, 32*(nh%4)) -> 4 column-tiles packed in the array
  evac (DVE): pa4 psum [128, 1024] f32 -> staging bf16, one CAST per group
  store (ACT hwdge ring): staging -> DRAM in 256 KiB per-group chunks
"""

import numpy as np
from contextlib import ExitStack

import ml_dtypes

import concourse.bacc as bacc
import concourse.tile as tile
import concourse.mybir as mybir
from concourse.bass_utils import run_bass_kernel_spmd

F32 = mybir.dt.float32
BF16 = mybir.dt.bfloat16
RELU = mybir.ActivationFunctionType.Relu
NP_BF16 = ml_dtypes.bfloat16

EPS = 1e-5
NCORES = 8

_cached_nc = None


def _build_program(n_vbd=6, at_bufs=5, G=4):
    nc = bacc.Bacc("TRN2", target_bir_lowering=False, debug=False)

    x_d = nc.dram_tensor("x_sh", [256, 16, 2048], BF16, kind="ExternalInput")
    at_d = nc.dram_tensor("at_sh", [16, 128, 16384], BF16, kind="ExternalInput")
    wc_d = nc.dram_tensor("w_prep", [256, 256], BF16, kind="ExternalInput")
    b_d = nc.dram_tensor("bias4", [128, 1024], F32, kind="ExternalInput")
    out_d = nc.dram_tensor("out_sh", [16, 128, 4096], BF16, kind="ExternalOutput")

    ngroups = 16 // G

    with tile.TileContext(nc) as tc, ExitStack() as ctx:
        const = ctx.enter_context(tc.tile_pool(name="const", bufs=1))
        xp = ctx.enter_context(tc.tile_pool(name="xp", bufs=3))
        atp = ctx.enter_context(tc.tile_pool(name="atp", bufs=at_bufs))
        vbdp = ctx.enter_context(tc.tile_pool(name="vbdp", bufs=1))
        tvp = ctx.enter_context(tc.tile_pool(name="tvp", bufs=3))
        stp = ctx.enter_context(tc.tile_pool(name="stp", bufs=3))
        pscp = ctx.enter_context(tc.tile_pool(name="pscp", bufs=2, space="PSUM"))
        psap = ctx.enter_context(tc.tile_pool(name="psap", bufs=2, space="PSUM"))

        w0 = const.tile([128, 256], BF16, name="w0")
        w1 = const.tile([128, 256], BF16, name="w1")
        nc.sync.dma_start(out=w0, in_=wc_d[0:128, :])
        nc.sync.dma_start(out=w1, in_=wc_d[128:256, :])
        bias4 = const.tile([128, 1024], F32, name="bias4_t")
        nc.sync.dma_start(out=bias4, in_=b_d[:, :])

        # Block-diagonal V4 tiles: columns = (pair 4, nh 16, win 2, d 16).
        # Zeroed once; relu writes only the diagonal cells (win0 -> rows 0:64
        # of win-0 columns, win1 -> rows 64:128 of win-1 columns), so zeros
        # persist across reuse and each V4[:, 512iG+32nh:+32] is exactly
        # block-diag(V0, V1) for pair iG head nh.
        vbd = []
        for i in range(n_vbd):
            t = vbdp.tile([128, 2048], BF16, tag=f"vbd{i}", name=f"vbd{i}")
            nc.vector.memset(t, 0.0)
            vbd.append(t)
        vbd_i = 0

        for hh in range(16):
            xt0 = xp.tile([128, 2048], BF16, tag="xt0", name=f"xt0_{hh}")
            xt1 = xp.tile([128, 2048], BF16, tag="xt1", name=f"xt1_{hh}")
            nc.sync.dma_start(out=xt0, in_=x_d[0:128, hh, :])
            nc.sync.dma_start(out=xt1, in_=x_d[128:256, hh, :])

            st = stp.tile([128, 4096], BF16, tag="st", name=f"st_{hh}")

            for g in range(ngroups):
                at = atp.tile([128, 1024 * G], BF16, tag="at", name=f"at_{hh}_{g}")
                nc.sync.dma_start(
                    out=at,
                    in_=at_d[hh, :, 1024 * G * g: 1024 * G * (g + 1)])

                # conv: 4 pairs into one grouped psum tile
                ps4 = pscp.tile([128, 1024], F32, tag="psc", name=f"ps_{hh}_{g}")
                for iG in range(G):
                    p8 = G * g + iG
                    xsl = slice(128 * p8, 128 * p8 + 128)
                    osl = slice(256 * iG, 256 * iG + 256)
                    nc.tensor.matmul(ps4[:, osl], xt0[:, xsl], w0,
                                     start=True, stop=False)
                    nc.tensor.matmul(ps4[:, osl], xt1[:, xsl], w1,
                                     start=False, stop=True)

                # bias add, one batched instr (DVE), psum -> sbuf f32
                tv4 = tvp.tile([128, 1024], F32, tag="tv", name=f"tv_{hh}_{g}")
                nc.vector.tensor_add(tv4, ps4, bias4)

                # relu into block-diag V4, one batched instr per window half
                V4 = vbd[vbd_i % n_vbd]
                vbd_i += 1
                V4r = V4.rearrange("p (iG nh two d) -> p iG nh two d",
                                   iG=4, nh=16, two=2, d=16)
                tv4r = tv4.rearrange("p (iG nh d) -> p iG nh d",
                                     iG=4, nh=16, d=16)
                nc.scalar.activation(V4r[0:64, :, :, 0, :], tv4r[0:64], RELU)
                nc.scalar.activation(V4r[64:128, :, :, 1, :], tv4r[64:128], RELU)

                # attention: 16 matmuls per pair into grouped psum
                pa4 = psap.tile([128, 1024], F32, tag="pa", name=f"pa_{hh}_{g}")
                for iG in range(G):
                    for j in range(4):
                        for quad in range(4):
                            nh = 4 * j + quad
                            nc.tensor.matmul(
                                pa4[32 * quad:32 * quad + 32,
                                    256 * iG + 64 * j: 256 * iG + 64 * j + 64],
                                V4[:, 512 * iG + 32 * nh: 512 * iG + 32 * nh + 32],
                                at[:, 1024 * iG + 64 * nh: 1024 * iG + 64 * nh + 64],
                                start=True, stop=True,
                                tile_position=(0, 32 * quad))

                # evac: one batched CAST (DVE), then store this 256KiB chunk
                nc.vector.tensor_copy(st[:, 1024 * g:1024 * (g + 1)], pa4)
                nc.scalar.dma_start(out=out_d[hh][:, 1024 * g:1024 * (g + 1)],
                                    in_=st[:, 1024 * g:1024 * (g + 1)])

    nc.compile()
    return nc


def _shard_inputs(x, attn_i, w_conv, bn_gamma, bn_beta, bn_mean, bn_var):
    inv_std = (bn_gamma / np.sqrt(bn_var + np.float32(EPS))).astype(np.float32)
    shift = (bn_beta - bn_mean * inv_std).astype(np.float32)
    bias4 = np.ascontiguousarray(
        np.broadcast_to(np.tile(shift, 4)[None, :], (128, 1024))).astype(np.float32)
    w_prep = np.ascontiguousarray((w_conv * inv_std[:, None]).T).astype(NP_BF16)
    x16 = x.astype(NP_BF16)
    at16 = attn_i.astype(NP_BF16)
    in_maps = []
    for core in range(NCORES):
        b, rh = core // 2, core % 2
        x_sh = x16[b, :, 128 * rh:128 * rh + 128, :]
        x_sh = np.ascontiguousarray(
            x_sh.reshape(256, 16, 8, 16, 2, 8).transpose(0, 1, 3, 4, 2, 5)
        ).reshape(256, 16, 2048)
        a_sl = at16[1024 * b + 512 * rh: 1024 * b + 512 * rh + 512]
        # [pair, 64win+k, 64nh+q], then partition-major per hh row
        # ([hh, p, pr, 1024]) so each at-load reads 8KiB/partition contiguous
        a_prep = a_sl.reshape(256, 2, 16, 64, 64).transpose(0, 1, 4, 2, 3) \
            .reshape(16, 16, 128, 1024)
        a_prep = np.ascontiguousarray(
            a_prep.transpose(0, 2, 1, 3)).reshape(16, 128, 16384)
        in_maps.append(dict(x_sh=x_sh, at_sh=a_prep, w_prep=w_prep,
                            bias4=bias4))
    return in_maps


def _unshard_output(results):
    out = np.empty((4, 256, 256, 256), np.float32)
    for core in range(NCORES):
        b, rh = core // 2, core % 2
        raw = np.asarray(results[core]["out_sh"]).astype(np.float32)
        # [hh, (quad, win, d), (pw, j, r1, r2)],  nh = 4j+quad
        r = raw.reshape(16, 4, 2, 16, 16, 4, 8, 8)
        # ch = 64j+16quad+d ; h = 8hh+r1 ; w = 16pw+8win+r2
        oc = r.transpose(5, 1, 3, 0, 6, 4, 2, 7).reshape(256, 128, 256)
        out[b, :, 128 * rh:128 * rh + 128, :] = oc
    return out


def get_program():
    global _cached_nc
    if _cached_nc is None:
        _cached_nc = _build_program()
    return _cached_nc


def run_sharded(in_maps, trace=False, **kwargs):
    nc = get_program()
    return run_bass_kernel_spmd(nc, in_maps, list(range(NCORES)),
                                trace=trace, **kwargs)


def kernel(x, attn_i, w_conv, bn_gamma, bn_beta, bn_mean, bn_var):
    x = np.asarray(x, dtype=np.float32)
    attn_i = np.asarray(attn_i, dtype=np.float32)
    w_conv = np.asarray(w_conv, dtype=np.float32)
    bn_gamma = np.asarray(bn_gamma, dtype=np.float32)
    bn_beta = np.asarray(bn_beta, dtype=np.float32)
    bn_mean = np.asarray(bn_mean, dtype=np.float32)
    bn_var = np.asarray(bn_var, dtype=np.float32)
    in_maps = _shard_inputs(x, attn_i, w_conv, bn_gamma, bn_beta, bn_mean, bn_var)
    res = run_sharded(in_maps)
    return _unshard_output(res.results)
